# revision 2
# baseline (speedup 1.0000x reference)
"""Trainium2 Bass kernel for nn_Critic (han 1008->2048->2048->512, q-MLP 520->2048->2048->1).

Data-parallel over 8 NeuronCores: batch 8192 -> 1024 rows/core, weights replicated.
Activations live in SBUF feature-major (x^T: [features, batch]); every layer is
out^T[f_tile, b_tile] += W[k_block, f_block].T @ x^T[k_block, b_tile].

Weights are whole-layer SBUF-resident bf16 slabs ([128, K, N] with dim1 = k-tile),
DMA'd in ~78 large transfers per iteration instead of ~414 small per-step slabs.
SBUF budget is met by ring-sharing weight buffers between layers whose lifetimes
don't overlap: {W1a, W3}, {W1b, Wm1e}, {W2h1, Wm2h1}, {W2h2, Wm2h2} each share one
buffer (the later layer's DMA waits for the earlier layer's last matmul read, which
the slack analysis covers with >5x margin). Matmuls keep (fi, b) inner order so
consecutive instructions share stationary weights (FWL reload elision).
Bias+ReLU fuse into one ScalarE activation per tile on the PSUM->SBUF path.
"""

import sys

sys.path.insert(0, "/opt/trn_rl_repo")

import numpy as np

N_CORES = 8
BATCH = 8192
B = BATCH // N_CORES  # rows per core
BT = 512              # batch tile = psum free dim
NB = B // BT          # batch tiles per core
FG = 2                # feature (output) 128-tiles per psum group; FG*NB banks/group

OBS_DIM = 1008
HAN_HIDDEN = 2048
HAN_OUT = 512
ACTION_DIM = 8
MLP_HIDDEN = 2048


def _k_tiles(K):
    """Split contraction dim K into 128-partition tiles."""
    sizes = []
    while K > 0:
        sizes.append(min(128, K))
        K -= 128
    return sizes


def _split_excess_waits(nc, max_waits=1):
    """Walrus codegen rejects instructions carrying more than ~1 embedded sync
    wait (notably fused 4-byte matmuls and NO-type control instructions).
    Move overflow waits onto same-engine single-wait NoOps inserted just
    before the instruction — the engine queue is in-order, so semantics are
    identical."""
    import concourse.mybir as mybir

    ctr = 0
    for func in nc.m.functions:
        for blk in func.blocks:
            out = []
            for inst in blk.instructions:
                si = inst.sync_info
                if si is not None and len(si.on_wait) > max_waits:
                    waits = list(si.on_wait)
                    while len(waits) > max_waits:
                        nop = mybir.InstNoOp(
                            name=f"waitsplit_nop_{ctr}", ins=[], outs=[])
                        ctr += 1
                        nop.engine = inst.engine
                        nop.bass_nofuse = True
                        nop.sync_info = mybir.SyncInfo(
                            on_wait=[waits.pop(0)], on_update=[])
                        nc.register_instruction(nop, overwrite=True)
                        out.append(nop)
                    inst.sync_info = mybir.SyncInfo(
                        on_wait=waits, on_update=list(si.on_update))
                out.append(inst)
            blk.instructions[:] = out
    return ctr


def _build(repeats=1, loop_n=None):
    import concourse.bass as bass
    import concourse.mybir as mybir
    import concourse.tile as tile

    f32 = mybir.dt.float32
    bf16 = mybir.dt.bfloat16
    Relu = mybir.ActivationFunctionType.Relu
    Ident = mybir.ActivationFunctionType.Identity

    nc = bass.Bass()

    # --- DRAM I/O (per-core shard shapes) ---
    obsT = nc.dram_tensor("obsT", [OBS_DIM, B], bf16, kind="ExternalInput")
    actT = nc.dram_tensor("actT", [ACTION_DIM, B], bf16, kind="ExternalInput")
    W1a = nc.dram_tensor("W1a", [512, HAN_HIDDEN], bf16, kind="ExternalInput")
    W1b = nc.dram_tensor("W1b", [OBS_DIM - 512, HAN_HIDDEN], bf16, kind="ExternalInput")
    W2h1 = nc.dram_tensor("W2h1", [1024, HAN_HIDDEN], bf16, kind="ExternalInput")
    W2h2 = nc.dram_tensor("W2h2", [1024, HAN_HIDDEN], bf16, kind="ExternalInput")
    W3 = nc.dram_tensor("W3", [HAN_HIDDEN, HAN_OUT], bf16, kind="ExternalInput")
    Wm1a = nc.dram_tensor("Wm1a", [ACTION_DIM, MLP_HIDDEN], bf16, kind="ExternalInput")
    Wm1e = nc.dram_tensor("Wm1e", [HAN_OUT, MLP_HIDDEN], bf16, kind="ExternalInput")
    Wm2h1 = nc.dram_tensor("Wm2h1", [1024, MLP_HIDDEN], bf16, kind="ExternalInput")
    Wm2h2 = nc.dram_tensor("Wm2h2", [1024, MLP_HIDDEN], bf16, kind="ExternalInput")
    wm3r = nc.dram_tensor("wm3r", [128, 16], bf16, kind="ExternalInput")
    b1r = nc.dram_tensor("b1r", [128, 16], f32, kind="ExternalInput")
    b2r = nc.dram_tensor("b2r", [128, 16], f32, kind="ExternalInput")
    b3r = nc.dram_tensor("b3r", [128, 4], f32, kind="ExternalInput")
    bm1r = nc.dram_tensor("bm1r", [128, 16], f32, kind="ExternalInput")
    bm2r = nc.dram_tensor("bm2r", [128, 16], f32, kind="ExternalInput")
    bm3r = nc.dram_tensor("bm3r", [1, 1], f32, kind="ExternalInput")
    qT = nc.dram_tensor("qT", [1, B], f32, kind="ExternalOutput")

    with tile.TileContext(nc) as tc:
        with (
            tc.tile_pool(name="acts", bufs=1) as acts,
            tc.tile_pool(name="wts", bufs=1) as wts,
            tc.tile_pool(name="bias", bufs=1) as bias_pool,
            tc.tile_pool(name="psum", bufs=8, space="PSUM") as psum_pool,
        ):
            # --- constants (once per exec) ---
            def _load_const(dram, shape, name, dt=f32):
                t = bias_pool.tile(shape, dt, tag=name, name=name)
                nc.sync.dma_start(t[:, :], dram[:, :])
                return t

            tb1 = _load_const(b1r, [128, 16], "tb1")
            tb2 = _load_const(b2r, [128, 16], "tb2")
            tb3 = _load_const(b3r, [128, 4], "tb3")
            tbm1 = _load_const(bm1r, [128, 16], "tbm1")
            tbm2 = _load_const(bm2r, [128, 16], "tbm2")
            tbm3 = _load_const(bm3r, [1, 1], "tbm3")
            twm3 = _load_const(wm3r, [128, 16], "twm3", dt=bf16)

            # --- input activations, feature-major ---
            obs_tiles = [
                acts.tile([128, B], bf16, tag="io", bufs=8, name=f"obs{k}")
                for k in range(len(_k_tiles(OBS_DIM)))
            ]
            act_tile = acts.tile([ACTION_DIM, B], bf16, tag="act", bufs=1,
                                 name="act")

            def load_inputs(sfx):
                for k, kp in enumerate(_k_tiles(OBS_DIM)):
                    nc.sync.dma_start(
                        obs_tiles[k][:kp, :], obsT[k * 128 : k * 128 + kp, :])
                nc.sync.dma_start(act_tile[:, :], actT[:, :])

            def load_slab(tag, name, dram, n_k, n_cols, k_sizes=None):
                """Whole-(half-)layer weight slab [128, n_k, n_cols]; one DMA
                per k-tile. tag-ring (bufs=1) shares the buffer with the other
                layer on the same tag."""
                t = wts.tile([128, n_k, n_cols], bf16, tag=tag, name=name)
                for j in range(n_k):
                    kp = 128 if k_sizes is None else k_sizes[j]
                    nc.sync.dma_start(
                        t[:kp, j, :], dram[j * 128 : j * 128 + kp, :])
                return t

            def layer(steps, n_f, bias_tile, func, out_tag, out_bufs,
                      out_name, rot=2):
                """steps: list of (lhsT_fn, x_tile, kp) where
                lhsT_fn(f) -> stationary AP [kp, 128] for output tile f.
                Returns n_f SBUF tiles [128, B] holding out^T (bf16)."""
                out_tiles = [
                    acts.tile([128, B], bf16, tag=out_tag, bufs=out_bufs,
                              name=f"{out_name}_{f}")
                    for f in range(n_f)
                ]
                r = rot % len(steps)
                steps = steps[r:] + steps[:r]
                total_k = len(steps)
                for fg in range(n_f // FG):
                    ps = [
                        psum_pool.tile([128, BT], f32, tag="ps",
                                       name=f"ps_{out_name}_{fg}_{i}")
                        for i in range(FG * NB)
                    ]
                    for step, (lhsT_fn, xt, kp) in enumerate(steps):
                        for fi in range(FG):
                            lhsT = lhsT_fn(fg * FG + fi)
                            for b in range(NB):
                                nc.tensor.matmul(
                                    ps[fi * NB + b][:, :],
                                    lhsT,
                                    xt[:kp, b * BT : (b + 1) * BT],
                                    start=(step == 0),
                                    stop=(step == total_k - 1),
                                )
                    for fi in range(FG):
                        f = fg * FG + fi
                        for b in range(NB):
                            nc.scalar.activation(
                                out_tiles[f][:, b * BT : (b + 1) * BT],
                                ps[fi * NB + b][:, :],
                                func,
                                bias=bias_tile[:, f : f + 1],
                            )
                return out_tiles

            def slab_steps(slab, x_tiles, k_sizes, x_base=0):
                """One step per k-tile of a resident slab."""
                steps = []
                for j, kp in enumerate(k_sizes):
                    def fn(f, slab=slab, j=j, kp=kp):
                        return slab[:kp, j, f * 128 : (f + 1) * 128]
                    steps.append((fn, x_tiles[x_base + j], kp))
                return steps

            def network(sfx, last):
                # Weight DMAs are placed so their tag-ring waits (on the
                # previous iteration's readers) resolve in SP program order.
                w1a = load_slab("wA", sfx + "w1a", W1a, 4, HAN_HIDDEN)
                w1b = load_slab("wB", sfx + "w1b", W1b, 4, HAN_HIDDEN,
                                k_sizes=_k_tiles(OBS_DIM - 512))
                w2h1 = load_slab("wC", sfx + "w2h1", W2h1, 8, HAN_HIDDEN)
                w2h2 = load_slab("wD", sfx + "w2h2", W2h2, 8, HAN_HIDDEN)
                wm1a_t = wts.tile([ACTION_DIM, MLP_HIDDEN], bf16, tag="wm1a",
                                  name=sfx + "wm1a")
                nc.sync.dma_start(wm1a_t[:, :], Wm1a[:, :])

                h1 = layer(
                    slab_steps(w1a, obs_tiles, [128] * 4)
                    + slab_steps(w1b, obs_tiles, _k_tiles(OBS_DIM - 512),
                                 x_base=4),
                    HAN_HIDDEN // 128, tb1, Relu, "big", 32, sfx + "h1")

                w3 = load_slab("wA", sfx + "w3", W3, 16, HAN_OUT)
                wm1e = load_slab("wB", sfx + "wm1e", Wm1e, 4, MLP_HIDDEN)

                h2 = layer(
                    slab_steps(w2h1, h1, [128] * 8)
                    + slab_steps(w2h2, h1, [128] * 8, x_base=8),
                    HAN_HIDDEN // 128, tb2, Relu, "big", 32, sfx + "h2")

                wm2h1 = load_slab("wC", sfx + "wm2h1", Wm2h1, 8, MLP_HIDDEN)
                wm2h2 = load_slab("wD", sfx + "wm2h2", Wm2h2, 8, MLP_HIDDEN)

                emb = layer(slab_steps(w3, h2, [128] * 16),
                            HAN_OUT // 128, tb3, Ident, "emb", 4, sfx + "emb")

                def wm1a_fn(f):
                    return wm1a_t[:ACTION_DIM, f * 128 : (f + 1) * 128]

                h3 = layer(
                    [(wm1a_fn, act_tile, ACTION_DIM)]
                    + slab_steps(wm1e, emb, [128] * 4),
                    MLP_HIDDEN // 128, tbm1, Relu, "big", 32, sfx + "h3")

                h4 = layer(
                    slab_steps(wm2h1, h3, [128] * 8)
                    + slab_steps(wm2h2, h3, [128] * 8, x_base=8),
                    MLP_HIDDEN // 128, tbm2, Relu, "big", 32, sfx + "h4")

                # --- final layer: q^T[1, B] = Wm3.T @ h4^T + bm3 ---
                q_sbuf = acts.tile([1, B], f32, tag="q", bufs=2,
                                   name=sfx + "q_sbuf")
                n_k6 = MLP_HIDDEN // 128
                k6_order = list(range(2, n_k6)) + [0, 1]
                for b in range(NB):
                    ps = psum_pool.tile([128, BT], f32, tag="ps",
                                        name=f"ps_{sfx}q_{b}")
                    for i, k in enumerate(k6_order):
                        nc.tensor.matmul(
                            ps[:1, :],
                            twm3[:, k : k + 1],
                            h4[k][:, b * BT : (b + 1) * BT],
                            start=(i == 0),
                            stop=(i == n_k6 - 1),
                        )
                    nc.scalar.activation(
                        q_sbuf[:1, b * BT : (b + 1) * BT],
                        ps[:1, :],
                        Ident,
                        bias=tbm3[:1, :1],
                    )
                if last:
                    nc.sync.dma_start(qT[:, :], q_sbuf[:1, :])

            if loop_n is not None:
                with tc.For_i(0, loop_n):
                    load_inputs("")
                    network("", True)
            else:
                for rep in range(repeats):
                    sfx = f"r{rep}_" if repeats > 1 else ""
                    load_inputs(sfx)
                    network(sfx, rep == repeats - 1)

    _split_excess_waits(nc)
    return nc


def make_in_maps(inputs):
    """Host-side sharding + layout massaging. Returns list of 8 per-core maps."""
    import ml_dtypes

    wdt = ml_dtypes.bfloat16
    obs = np.asarray(inputs["obs"], dtype=np.float32)
    action = np.asarray(inputs["action"], dtype=np.float32)
    W1 = np.asarray(inputs["W1"], np.float32)
    W2 = np.asarray(inputs["W2"], np.float32)
    Wm1 = np.asarray(inputs["Wm1"], np.float32)
    Wm2 = np.asarray(inputs["Wm2"], np.float32)

    def c(a):
        return np.ascontiguousarray(a.astype(wdt))

    shared = {
        "W1a": c(W1[:512]),
        "W1b": c(W1[512:]),
        "W2h1": c(W2[:1024]),
        "W2h2": c(W2[1024:]),
        "W3": c(np.asarray(inputs["W3"], np.float32)),
        "Wm1a": c(Wm1[:ACTION_DIM]),
        "Wm1e": c(Wm1[ACTION_DIM:]),
        "Wm2h1": c(Wm2[:1024]),
        "Wm2h2": c(Wm2[1024:]),
        "wm3r": c(np.asarray(inputs["Wm3"], np.float32).reshape(16, 128).T),
        "b1r": np.ascontiguousarray(
            np.asarray(inputs["b1"], np.float32).reshape(16, 128).T),
        "b2r": np.ascontiguousarray(
            np.asarray(inputs["b2"], np.float32).reshape(16, 128).T),
        "b3r": np.ascontiguousarray(
            np.asarray(inputs["b3"], np.float32).reshape(4, 128).T),
        "bm1r": np.ascontiguousarray(
            np.asarray(inputs["bm1"], np.float32).reshape(16, 128).T),
        "bm2r": np.ascontiguousarray(
            np.asarray(inputs["bm2"], np.float32).reshape(16, 128).T),
        "bm3r": np.asarray(inputs["bm3"], np.float32).reshape(1, 1),
    }
    in_maps = []
    for core in range(N_CORES):
        sl = slice(core * B, (core + 1) * B)
        m = dict(shared)
        m["obsT"] = c(obs[sl].T)
        m["actT"] = c(action[sl].T)
        in_maps.append(m)
    return in_maps


def run_sharded(inputs):
    """Build + run on 8 cores; returns per-core results list."""
    from concourse.bass_utils import run_bass_kernel_spmd

    nc = _build()
    in_maps = make_in_maps(inputs)
    res = run_bass_kernel_spmd(nc, in_maps, core_ids=list(range(N_CORES)))
    return res


def kernel(**inputs):
    res = run_sharded(inputs)
    q = np.empty((BATCH, 1), np.float32)
    for c in range(N_CORES):
        q[c * B : (c + 1) * B, 0] = res.results[c]["qT"][0]
    return q
